# revision 1
# baseline (speedup 1.0000x reference)
"""Trainium2 Bass kernel for causal multi-head attention.

Problem: B=2, T=4096, D=768, H=12 heads, d_k=64, causal mask.
Sharding: 8 cores = 2 batches x 4 head-groups (3 heads each).
Each core computes its batch's qkv projection (its heads only), flash-style
attention with transposed scores (S^T = k q^T, so softmax statistics land in
the matmul-friendly layout with no P-transposes), and a partial output
projection. Host sums the 4 head-group partials per batch and adds the
folded bias constant (v-bias @ W_out + b_out). The k-bias is dropped
entirely (softmax is invariant to per-query score shifts).

Self-contained: hardcodes all shapes; only imports the concourse runtime.
"""

import os
import sys

sys.path.insert(0, "/opt/trn_rl_repo")

from contextlib import ExitStack

import numpy as np

import concourse.bass as bass
import concourse.mybir as mybir
import concourse.tile as tile
from concourse import bacc
from concourse.bass_utils import run_bass_kernel_spmd

F32 = mybir.dt.float32

B, T, D = 2, 4096, 768
H, DK = 12, 64
HPC = 3          # heads per core
N_CORES = 8
ICH_W = 512      # i-chunk width (queries per outer step)
JB_W = 128       # j-block width (keys per matmul)

USE_DMA_TRANSPOSE = False   # fp32 xbar DMA-transpose unsupported (2-byte only)
USE_F32R = True             # run matmuls in float32r (1 cyc/row at N>=256 vs 4 for fp32)
F32R = mybir.dt.float32r
VPAD = 256                  # pad v-projection rhs to 256 cols so f32r hits fast path


MDT = F32R if USE_F32R else F32     # dtype for matmul operand tiles


def _r(ap):
    return ap


def build_program(t=T):
    """Build the SPMD Bass program for one core (all cores identical)."""
    n_ich = t // ICH_W          # i-chunks
    n_tch = t // 128            # t-chunks of 128 tokens
    KT = D // 128               # 6 contraction tiles for the projections

    nc = bacc.Bacc("TRN2", target_bir_lowering=False, debug=False,
                   num_devices=N_CORES)

    x_d = nc.dram_tensor("x", [t, D], F32, kind="ExternalInput").ap()
    # qk projection weights, 4 chunks of 128 output channels:
    # ch0=[q1|q2] ch1=[k1|k2] ch2=[q3|k3] ch3=[k3|q3]
    wqk_d = nc.dram_tensor("wqk", [D, 512], F32, kind="ExternalInput").ap()
    bqk_d = nc.dram_tensor("bqk", [512], F32, kind="ExternalInput").ap()
    wv_d = nc.dram_tensor("wv", [D, VPAD], F32, kind="ExternalInput").ap()
    wout_d = nc.dram_tensor("wout", [HPC * DK, D], F32, kind="ExternalInput").ap()
    out_d = nc.dram_tensor("out", [t, D], F32, kind="ExternalOutput").ap()

    with tile.TileContext(nc) as tc, ExitStack() as top:
        consts = top.enter_context(tc.tile_pool(name="consts", bufs=1))
        # persistent activations
        persist = top.enter_context(tc.tile_pool(name="persist", bufs=1))

        # q^T / k^T per chunk: [128, 4, t]
        qk_sb = persist.tile([128, 4, t], MDT)
        # v (natural layout) + ones column: [128, n_tch, HPC, 65]
        vaug_sb = persist.tile([128, n_tch, HPC, DK + 1], MDT)

        wqk_st = consts.tile([128, KT, 512], F32)
        nc.sync.dma_start(out=wqk_st, in_=wqk_d.rearrange("(kt p) c -> p kt c", p=128))
        wqk_sb = consts.tile([128, KT, 512], MDT)
        nc.vector.tensor_copy(wqk_sb, wqk_st)
        bqk_sb = consts.tile([128, 4], F32)
        nc.sync.dma_start(out=bqk_sb, in_=bqk_d.rearrange("(ch p) -> p ch", p=128))
        wv_st = consts.tile([128, KT, VPAD], F32)
        nc.sync.dma_start(out=wv_st, in_=wv_d.rearrange("(kt p) c -> p kt c", p=128))
        wv_sb = consts.tile([128, KT, VPAD], MDT)
        nc.vector.tensor_copy(wv_sb, wv_st)
        wout_st = consts.tile([64, HPC, D], F32)
        nc.sync.dma_start(out=wout_st, in_=wout_d.rearrange("(h p) m -> p h m", p=64))
        wout_sb = consts.tile([64, HPC, D], MDT)
        nc.vector.tensor_copy(wout_sb, wout_st)

        ones3 = consts.tile([128, 3], F32)
        nc.vector.memset(ones3, 1.0)

        identity = None
        if not USE_DMA_TRANSPOSE:
            from concourse.masks import make_identity
            identity = consts.tile([128, 128], F32)
            make_identity(nc, identity)

        # ---------------- Phase 1+2: x^T (streamed) + projections ----------
        with tc.tile_pool(name="xt", bufs=3) as xtp, \
             tc.tile_pool(name="xn", bufs=3) as xnp, \
             tc.tile_pool(name="p2ps", bufs=2, space="PSUM") as p2ps, \
             tc.tile_pool(name="p2ps_v", bufs=2, space="PSUM") as p2psv:
            for ich in range(n_ich):
                i0 = ich * ICH_W
                xt = xtp.tile([128, KT, ICH_W], MDT, tag="xt")
                if USE_DMA_TRANSPOSE:
                    for c in range(D // 64):
                        nc.sync.dma_start(
                            out=xt[(c % 2) * 64:(c % 2) * 64 + 64, c // 2, :],
                            in_=x_d[i0:i0 + ICH_W, c * 64:(c + 1) * 64],
                            transpose=True,
                        )
                else:
                    for tl in range(ICH_W // 128):
                        xn = xnp.tile([128, D], F32, tag="xn")
                        nc.sync.dma_start(
                            out=xn, in_=x_d[i0 + tl * 128:i0 + (tl + 1) * 128, :])
                        for kt in range(KT):
                            tps = p2ps.tile([128, 128], F32, tag="tr",
                                            space="PSUM")
                            nc.tensor.transpose(
                                tps, xn[:, kt * 128:(kt + 1) * 128], identity)
                            nc.vector.tensor_copy(
                                xt[:, kt, tl * 128:(tl + 1) * 128], tps)
                # q^T/k^T chunks for this i-range
                for ch in range(4):
                    qps = p2ps.tile([128, ICH_W], F32, tag="qk", space="PSUM")
                    for kt in range(KT):
                        nc.tensor.matmul(
                            qps,
                            lhsT=_r(wqk_sb[:, kt, ch * 128:(ch + 1) * 128]),
                            rhs=_r(xt[:, kt, :]),
                            start=(kt == 0), stop=(kt == KT - 1),
                        )
                    nc.vector.tensor_scalar_add(
                        qk_sb[:, ch, i0:i0 + ICH_W], qps, bqk_sb[:, ch:ch + 1])
                # v natural for the 4 t-chunks in this i-range
                for tl in range(ICH_W // 128):
                    tch = ich * (ICH_W // 128) + tl
                    vps = p2psv.tile([128, VPAD], F32, tag="v", space="PSUM")
                    for kt in range(KT):
                        nc.tensor.matmul(
                            vps,
                            lhsT=_r(xt[:, kt, tl * 128:(tl + 1) * 128]),
                            rhs=_r(wv_sb[:, kt, :]),
                            start=(kt == 0), stop=(kt == KT - 1),
                        )
                    nc.vector.tensor_copy(
                        vaug_sb[:, tch, :, 0:DK],
                        vps[:, 0:HPC * DK].rearrange("p (h d) -> p h d", h=HPC),
                    )
                    nc.vector.tensor_copy(
                        vaug_sb[:, tch, :, DK:DK + 1],
                        ones3.rearrange("p (a b) -> p a b", b=1))

        # head views: (qT, kT) partition slices + base partition for pairing
        # h0: q=ch0[0:64]   k=ch1[0:64]    (base 0)
        # h1: q=ch0[64:128] k=ch1[64:128]  (base 64)
        # h2 even jb: q=ch2[0:64]  k=ch3[0:64]   (base 0)
        # h2 odd  jb: q=ch3[64:128] k=ch2[64:128] (base 64)

        # ---------------- Phase 3: attention + out projection ---------------
        with tc.tile_pool(name="stps", bufs=2, space="PSUM") as stps, \
             tc.tile_pool(name="cps", bufs=2, space="PSUM") as cpsp, \
             tc.tile_pool(name="ops", bufs=1, space="PSUM") as opsp, \
             tc.tile_pool(name="pt", bufs=3) as ptp, \
             tc.tile_pool(name="ctxn", bufs=3) as ctxp, \
             tc.tile_pool(name="small", bufs=4) as smp, \
             tc.tile_pool(name="outsb", bufs=2) as outp:
            for ich in range(n_ich):
                i0 = ich * ICH_W
                njb = (i0 + ICH_W) // JB_W     # causal: j-blocks 0..njb-1
                ctxn = {}

                # ---- pass A: heads 0 and 1, row-group paired ----
                cps0 = cpsp.tile([65, ICH_W], F32, tag="cps", space="PSUM")
                cps1 = cpsp.tile([65, ICH_W], F32, tag="cps", space="PSUM")
                for jb in range(njb):           # 1 j-block x 2 heads per group
                    j0 = jb * JB_W
                    st = stps.tile([128, 2, ICH_W], F32, tag="st", space="PSUM")
                    # h0 at rows 0-63, h1 at rows 64-127: concurrent MMs
                    nc.tensor.matmul(
                        st[:, 0, :],
                        lhsT=_r(qk_sb[0:64, 1, j0:j0 + JB_W]),
                        rhs=_r(qk_sb[0:64, 0, i0:i0 + ICH_W]),
                        start=True, stop=True)
                    nc.tensor.matmul(
                        st[:, 1, :],
                        lhsT=_r(qk_sb[64:128, 1, j0:j0 + JB_W]),
                        rhs=_r(qk_sb[64:128, 0, i0:i0 + ICH_W]),
                        start=True, stop=True)
                    pt = ptp.tile([128, 2, ICH_W], MDT, tag="pt")
                    nc.scalar.activation(pt, st,
                                         mybir.ActivationFunctionType.Exp,
                                         bias=0.0, scale=1.0 / np.sqrt(DK))
                    s = jb - (njb - 4)          # diag position if >= 0
                    if s >= 0:
                        w = 128 * (s + 1)
                        for hh in range(2):
                            nc.gpsimd.affine_select(
                                out=pt[:, hh, 0:w],
                                in_=pt[:, hh, 0:w],
                                compare_op=mybir.AluOpType.is_ge,
                                fill=0.0, base=-128 * s,
                                pattern=[[1, w]], channel_multiplier=-1)
                    nc.tensor.matmul(
                        cps0, lhsT=_r(vaug_sb[:, jb, 0, :]),
                        rhs=_r(pt[:, 0, :]),
                        start=(jb == 0), stop=(jb == njb - 1))
                    nc.tensor.matmul(
                        cps1, lhsT=_r(vaug_sb[:, jb, 1, :]),
                        rhs=_r(pt[:, 1, :]),
                        start=(jb == 0), stop=(jb == njb - 1))

                # ---- normalize h0/h1 now so their cps slots free before
                # pass B allocates cps2 (cps pool has bufs=2) ----
                for h, cps in ((0, cps0), (1, cps1)):
                    recip = smp.tile([1, ICH_W], F32, tag="recip")
                    nc.vector.reciprocal(recip, cps[64:65, :])
                    rb = smp.tile([64, ICH_W], F32, tag="rb")
                    nc.gpsimd.partition_broadcast(rb, recip)
                    cn = ctxp.tile([64, ICH_W], MDT, tag="ctxn")
                    nc.vector.tensor_mul(cn, cps[0:64, :], rb)
                    ctxn[h] = cn

                # ---- pass B: head 2, alternating row groups ----
                cps2 = cpsp.tile([65, ICH_W], F32, tag="cps", space="PSUM")
                for grp in range(njb // 2):     # 2 j-blocks per psum group
                    st = stps.tile([128, 2, ICH_W], F32, tag="st", space="PSUM")
                    for jj in range(2):
                        jb = grp * 2 + jj
                        j0 = jb * JB_W
                        if jb % 2 == 0:
                            lhsT = qk_sb[0:64, 3, j0:j0 + JB_W]
                            rhs = qk_sb[0:64, 2, i0:i0 + ICH_W]
                        else:
                            lhsT = qk_sb[64:128, 2, j0:j0 + JB_W]
                            rhs = qk_sb[64:128, 3, i0:i0 + ICH_W]
                        nc.tensor.matmul(st[:, jj, :], lhsT=_r(lhsT),
                                         rhs=_r(rhs), start=True, stop=True)
                    pt = ptp.tile([128, 2, ICH_W], MDT, tag="pt")
                    nc.scalar.activation(pt, st,
                                         mybir.ActivationFunctionType.Exp,
                                         bias=0.0, scale=1.0 / np.sqrt(DK))
                    for jj in range(2):
                        jb = grp * 2 + jj
                        s = jb - (njb - 4)
                        if s >= 0:
                            w = 128 * (s + 1)
                            nc.gpsimd.affine_select(
                                out=pt[:, jj, 0:w], in_=pt[:, jj, 0:w],
                                compare_op=mybir.AluOpType.is_ge,
                                fill=0.0, base=-128 * s,
                                pattern=[[1, w]], channel_multiplier=-1)
                    for jj in range(2):
                        jb = grp * 2 + jj
                        nc.tensor.matmul(
                            cps2, lhsT=_r(vaug_sb[:, jb, 2, :]),
                            rhs=_r(pt[:, jj, :]),
                            start=(jb == 0), stop=(jb == njb - 1))

                # ---- normalize head 2 ----
                for h, cps in ((2, cps2),):
                    recip = smp.tile([1, ICH_W], F32, tag="recip")
                    nc.vector.reciprocal(recip, cps[64:65, :])
                    rb = smp.tile([64, ICH_W], F32, tag="rb")
                    nc.gpsimd.partition_broadcast(rb, recip)
                    cn = ctxp.tile([64, ICH_W], MDT, tag="ctxn")
                    nc.vector.tensor_mul(cn, cps[0:64, :], rb)
                    ctxn[h] = cn

                # ---- partial out projection for this i-chunk ----
                for tsub in range(ICH_W // 128):
                    ops = opsp.tile([128, D], F32, tag="ops", space="PSUM")
                    for h in range(HPC):
                        for mi, (m0, m1) in enumerate(((0, 512), (512, D))):
                            nc.tensor.matmul(
                                ops[:, m0:m1],
                                lhsT=_r(ctxn[h][:, tsub * 128:(tsub + 1) * 128]),
                                rhs=_r(wout_sb[:, h, m0:m1]),
                                start=(h == 0), stop=(h == HPC - 1))
                    osb = outp.tile([128, D], F32, tag="osb")
                    nc.vector.tensor_copy(osb, ops)
                    nc.sync.dma_start(
                        out=out_d[i0 + tsub * 128:i0 + (tsub + 1) * 128, :],
                        in_=osb)

    nc.compile()
    return nc


def make_core_inputs(x_b, W_qkv, b_qkv, W_out, hg):
    """Host-side weight slicing/permutation for one head-group hg (0..3)."""
    heads = [hg * HPC + i for i in range(HPC)]
    # W_qkv last-dim layout: c = h*192 + s*64 + d  (s: 0=q 1=k 2=v)
    def cols(h, s):
        return slice(h * 192 + s * 64, h * 192 + s * 64 + 64)

    q = [np.asarray(W_qkv[:, cols(h, 0)]) for h in heads]
    k = [np.asarray(W_qkv[:, cols(h, 1)]) for h in heads]
    v = [np.asarray(W_qkv[:, cols(h, 2)]) for h in heads]
    bq = [np.asarray(b_qkv[cols(h, 0)]) for h in heads]

    wqk = np.concatenate([q[0], q[1], k[0], k[1], q[2], k[2], k[2], q[2]],
                         axis=1).astype(np.float32)
    z = np.zeros(64, np.float32)
    bqk = np.concatenate([bq[0], bq[1], z, z, bq[2], z, z, bq[2]]).astype(
        np.float32)
    wv = np.concatenate(v, axis=1).astype(np.float32)
    wv = np.pad(wv, ((0, 0), (0, 256 - wv.shape[1])))
    wout = np.concatenate(
        [np.asarray(W_out[h * DK:(h + 1) * DK, :]) for h in heads],
        axis=0).astype(np.float32)
    return {
        "x": np.ascontiguousarray(np.asarray(x_b, np.float32)),
        "wqk": np.ascontiguousarray(wqk),
        "bqk": np.ascontiguousarray(bqk),
        "wv": np.ascontiguousarray(wv),
        "wout": np.ascontiguousarray(wout),
    }


_CACHE = {}


def _get_program(t=T):
    if t not in _CACHE:
        _CACHE[t] = build_program(t)
    return _CACHE[t]


def run_cores(inputs, t=T, trace=False):
    nc = _get_program(t)
    x = np.asarray(inputs["x"], np.float32)
    in_maps = []
    for core in range(N_CORES):
        b, hg = core // 4, core % 4
        in_maps.append(make_core_inputs(x[b], inputs["W_qkv"],
                                        inputs["b_qkv"], inputs["W_out"], hg))
    res = run_bass_kernel_spmd(nc, in_maps, list(range(N_CORES)), trace=trace)
    return res


def gather(inputs, results):
    b_qkv = np.asarray(inputs["b_qkv"], np.float32)
    W_out = np.asarray(inputs["W_out"], np.float32)
    b_out = np.asarray(inputs["b_out"], np.float32)
    bv = np.concatenate([b_qkv[h * 192 + 128:h * 192 + 192] for h in range(H)])
    fold = bv @ W_out + b_out                      # [D]
    t = results[0]["out"].shape[0]
    out = np.zeros((B, t, D), np.float32)
    for core in range(N_CORES):
        out[core // 4] += results[core]["out"]
    out += fold[None, None, :]
    return out


def kernel(**inputs):
    res = run_cores(inputs)
    return gather(inputs, res.results)


if __name__ == "__main__":
    # smoke test with random data
    rng = np.random.default_rng(0)
    inputs = {
        "x": rng.standard_normal((B, T, D), dtype=np.float32),
        "mask": np.triu(np.ones((T, T), dtype=bool), k=1),
        "W_qkv": (rng.standard_normal((D, 3 * D), dtype=np.float32)
                  / np.sqrt(D)),
        "b_qkv": rng.standard_normal(3 * D, dtype=np.float32) * 0.02,
        "W_out": (rng.standard_normal((D, D), dtype=np.float32)
                  / np.sqrt(D)),
        "b_out": rng.standard_normal(D, dtype=np.float32) * 0.02,
    }
    out = kernel(**inputs)
    print(out.shape, out.dtype)



# revision 3
# speedup vs baseline: 1.2616x; 1.2616x over previous
"""Trainium2 Bass kernel for causal multi-head attention.

Problem: B=2, T=4096, D=768, H=12 heads, d_k=64, causal mask.
Sharding: 8 cores = 2 batches x 4 head-groups (3 heads each).

v2 design:
- All inputs shipped fp16 (x pre-transposed on host into [6, 128, T'] chunks).
- x is uploaded T-sharded (each core gets a distinct 1024-token slice of its
  batch) and AllGather'd on-device across the 4-core batch group, cutting
  host->device x bytes 4x vs replicating.
- All matmuls run fp16 operands (1 cyc/row + fast weight load), fp32 PSUM.
- Attention uses transposed scores (S^T = k q^T) so softmax statistics land
  matmul-friendly with no P-transposes; denominators via an appended
  ones-column in v (row 64 of the PV accumulation).
- Softmax normalization via reciprocal_approx_fast on a broadcast tile
  (the old single-lane PSUM reciprocal was 3.3us per call).
- Projections for chunk i+1 and the deferred output projection of chunk i-1
  are interleaved into attention's matmul stream as filler, keeping TensorE
  dense so the HAM clock gate stays at full rate.
- Partial outputs are ReduceScatter'd on-device (fp16) per 512-token chunk,
  overlapped with compute; each core returns a disjoint [8, 128, 768] slab,
  cutting device->host bytes 16x vs fp32 replicated partials.
- Host folds the v-bias through W_out and adds b_out. k-bias is dropped
  (softmax is invariant to per-query score shifts).

Self-contained: hardcodes all shapes; only imports the concourse runtime.
"""

import sys

sys.path.insert(0, "/opt/trn_rl_repo")

from contextlib import ExitStack

import numpy as np

import concourse.bass as bass
import concourse.mybir as mybir
import concourse.tile as tile
from concourse import bacc
from concourse.bass_utils import run_bass_kernel_spmd

F16 = mybir.dt.float16
F32 = mybir.dt.float32

B, T, D = 2, 4096, 768
H, DK = 12, 64
HPC = 3          # heads per core
N_CORES = 8
ICH_W = 512      # i-chunk width (queries per outer step)
JB_W = 128       # j-block width (keys per matmul)
KT = D // 128    # contraction tiles for projections
VW = HPC * DK    # v projection width
GROUPS = [[0, 1, 2, 3], [4, 5, 6, 7]]

USE_COLL = True  # AllGather x + ReduceScatter out on-device


def build_program(t=T, use_coll=USE_COLL):
    """Build the SPMD Bass program for one core (all cores identical)."""
    n_ich = t // ICH_W
    n_tch = t // 128
    tpc = t // 4                  # tokens per core in the x shard (AG mode)

    nc = bacc.Bacc("TRN2", target_bir_lowering=False, debug=False,
                   num_devices=N_CORES)

    xin_t = tpc if use_coll else t
    x_d = nc.dram_tensor("x", [KT, 128, xin_t], F16, kind="ExternalInput").ap()
    # qk projection weights, 4 chunks of 128 output channels:
    # ch0=[q0|q1] ch1=[k0|k1] ch2=[q2|k2] ch3=[k2|q2]
    wqk_d = nc.dram_tensor("wqk", [D, 512], F16, kind="ExternalInput").ap()
    bqk_d = nc.dram_tensor("bqk", [512], F32, kind="ExternalInput").ap()
    wv_d = nc.dram_tensor("wv", [D, VW], F16, kind="ExternalInput").ap()
    wout_d = nc.dram_tensor("wout", [VW, D], F16, kind="ExternalInput").ap()
    if use_coll:
        out_d = nc.dram_tensor("out", [n_ich, JB_W, D], F16,
                               kind="ExternalOutput").ap()
    else:
        out_d = nc.dram_tensor("out", [t, D], F16, kind="ExternalOutput").ap()

    with tile.TileContext(nc) as tc, ExitStack() as top:
        consts = top.enter_context(tc.tile_pool(name="consts", bufs=1))
        persist = top.enter_context(tc.tile_pool(name="persist", bufs=1))

        wqk_sb = consts.tile([128, KT, 512], F16)
        nc.sync.dma_start(out=wqk_sb,
                          in_=wqk_d.rearrange("(kt p) c -> p kt c", p=128))
        bqk_sb = consts.tile([128, 4], F32)
        nc.sync.dma_start(out=bqk_sb, in_=bqk_d.rearrange("(ch p) -> p ch",
                                                          p=128))
        wv_sb = consts.tile([128, KT, VW], F16)
        nc.sync.dma_start(out=wv_sb,
                          in_=wv_d.rearrange("(kt p) c -> p kt c", p=128))
        wout_sb = consts.tile([64, HPC, D], F16)
        nc.sync.dma_start(out=wout_sb,
                          in_=wout_d.rearrange("(h p) m -> p h m", p=64))

        # persistent activations: q^T/k^T chunks and v (+ones col)
        qk_sb = persist.tile([128, 4, t], F16)
        vaug_sb = persist.tile([128, n_tch, HPC, DK + 1], F16)
        nc.vector.memset(vaug_sb[:, :, :, DK:DK + 1], 1.0)

        if use_coll:
            dram = top.enter_context(
                tc.tile_pool(name="dram", bufs=1, space="DRAM"))
            xb = dram.tile([KT, 128, tpc], F16)
            xg = dram.tile([4, KT, 128, tpc], F16)
            ob = dram.tile([t, D], F16)
            rsb = dram.tile([n_ich, JB_W, D], F16)
            nc.gpsimd.dma_start(xb[:], x_d)
            nc.gpsimd.collective_compute(
                "AllGather", mybir.AluOpType.bypass, replica_groups=GROUPS,
                ins=[xb.opt()], outs=[xg.opt()])

        with tc.tile_pool(name="xtp", bufs=2) as xtp, \
             tc.tile_pool(name="stps", bufs=2, space="PSUM") as stps, \
             tc.tile_pool(name="cpsp", bufs=3, space="PSUM") as cpsp, \
             tc.tile_pool(name="fillp", bufs=1, space="PSUM") as fillp, \
             tc.tile_pool(name="ptp", bufs=3) as ptp, \
             tc.tile_pool(name="ctxp", bufs=6) as ctxp, \
             tc.tile_pool(name="smp", bufs=4) as smp, \
             tc.tile_pool(name="outp", bufs=2) as outp:

            def load_xt(j):
                xt = xtp.tile([128, KT, ICH_W], F16, tag="xt")
                i0 = j * ICH_W
                if use_coll:
                    r, off = divmod(i0, tpc)
                    src = xg[r, :, :, off:off + ICH_W].rearrange(
                        "kt p w -> p kt w")
                else:
                    src = x_d[:, :, i0:i0 + ICH_W].rearrange("kt p w -> p kt w")
                nc.sync.dma_start(out=xt, in_=src)
                return xt

            def proj_thunks(j, xt):
                """Filler thunks computing chunk j's qkv projections."""
                i0 = j * ICH_W
                ths = []
                for ch in range(4):
                    def th(ch=ch):
                        qps = fillp.tile([128, 512], F32, tag="fill",
                                         space="PSUM")
                        for kt in range(KT):
                            nc.tensor.matmul(
                                qps,
                                lhsT=wqk_sb[:, kt, ch * 128:(ch + 1) * 128],
                                rhs=xt[:, kt, :],
                                start=(kt == 0), stop=(kt == KT - 1))
                        nc.vector.tensor_scalar_add(
                            qk_sb[:, ch, i0:i0 + ICH_W], qps,
                            bqk_sb[:, ch:ch + 1])
                    ths.append(th)
                for tl in range(4):
                    def th(tl=tl):
                        vps = fillp.tile([128, 512], F32, tag="fill",
                                         space="PSUM")
                        for kt in range(KT):
                            nc.tensor.matmul(
                                vps[:, 0:VW],
                                lhsT=xt[:, kt, tl * 128:(tl + 1) * 128],
                                rhs=wv_sb[:, kt, :],
                                start=(kt == 0), stop=(kt == KT - 1))
                        nc.vector.tensor_copy(
                            vaug_sb[:, j * 4 + tl, :, 0:DK],
                            vps[:, 0:VW].rearrange("p (h d) -> p h d", h=HPC))
                    ths.append(th)
                return ths

            def outproj_thunks(j, ctxn):
                """Filler thunks for chunk j's output projection (+ its RS)."""
                i0 = j * ICH_W
                ths = []
                osbs = {}
                for tsub in range(4):
                    for mh, (m0, m1) in enumerate(((0, 384), (384, D))):
                        def th(tsub=tsub, mh=mh, m0=m0, m1=m1):
                            if mh == 0:
                                osbs[tsub] = outp.tile([128, D], F16,
                                                       tag="osb", name="osb")
                            osb = osbs[tsub]
                            ops = fillp.tile([128, 512], F32, tag="fill",
                                             space="PSUM")
                            for h in range(HPC):
                                nc.tensor.matmul(
                                    ops[:, 0:384],
                                    lhsT=ctxn[h][:,
                                                 tsub * 128:(tsub + 1) * 128],
                                    rhs=wout_sb[:, h, m0:m1],
                                    start=(h == 0), stop=(h == HPC - 1))
                            nc.vector.tensor_copy(osb[:, m0:m1], ops[:, 0:384])
                            if mh == 1:
                                dst = ob if use_coll else out_d
                                nc.sync.dma_start(
                                    out=dst[i0 + tsub * 128:
                                            i0 + (tsub + 1) * 128, :],
                                    in_=osb)
                        ths.append(th)
                if use_coll:
                    def th_rs(j=j):
                        nc.gpsimd.collective_compute(
                            "ReduceScatter", mybir.AluOpType.add,
                            replica_groups=GROUPS,
                            ins=[ob[j * ICH_W:(j + 1) * ICH_W, :].opt()],
                            outs=[rsb[j].opt()])
                        nc.gpsimd.dma_start(out_d[j], rsb[j])
                    ths.append(th_rs)
                return ths

            def normalize(cps):
                den = smp.tile([1, ICH_W], F32, tag="den")
                nc.vector.tensor_copy(den, cps[DK:DK + 1, :])
                rec = smp.tile([1, ICH_W], F32, tag="rec")
                nc.vector.reciprocal_approx_fast(out=rec, in_=den)
                rb = smp.tile([64, ICH_W], F32, tag="rb")
                nc.gpsimd.partition_broadcast(rb, rec)
                cn = ctxp.tile([64, ICH_W], F16, tag="ctxn")
                nc.vector.tensor_mul(cn, cps[0:DK, :], rb)
                return cn

            pending = []

            def drain(k):
                for _ in range(k):
                    if pending:
                        pending.pop(0)()

            xt = load_xt(0)
            for th in proj_thunks(0, xt):
                th()

            for ich in range(n_ich):
                i0 = ich * ICH_W
                njb = (i0 + ICH_W) // JB_W
                if ich + 1 < n_ich:
                    xt = load_xt(ich + 1)
                    pending.extend(proj_thunks(ich + 1, xt))

                # ---- pass A: heads 0 and 1, row-group paired ----
                cps0 = cpsp.tile([DK + 1, ICH_W], F32, tag="cps", space="PSUM")
                cps1 = cpsp.tile([DK + 1, ICH_W], F32, tag="cps", space="PSUM")
                for jb in range(njb):
                    j0 = jb * JB_W
                    st = stps.tile([128, 2, ICH_W], F32, tag="st",
                                   space="PSUM")
                    nc.tensor.matmul(
                        st[:, 0, :], lhsT=qk_sb[0:64, 1, j0:j0 + JB_W],
                        rhs=qk_sb[0:64, 0, i0:i0 + ICH_W],
                        start=True, stop=True)
                    nc.tensor.matmul(
                        st[:, 1, :], lhsT=qk_sb[64:128, 1, j0:j0 + JB_W],
                        rhs=qk_sb[64:128, 0, i0:i0 + ICH_W],
                        start=True, stop=True)
                    pt = ptp.tile([128, 2, ICH_W], F16, tag="pt")
                    nc.scalar.activation(pt, st,
                                         mybir.ActivationFunctionType.Exp,
                                         bias=0.0, scale=1.0 / np.sqrt(DK))
                    s = jb - (njb - 4)          # diag position if >= 0
                    if s >= 0:
                        w = 128 * (s + 1)
                        for hh in range(2):
                            nc.gpsimd.affine_select(
                                out=pt[:, hh, 0:w], in_=pt[:, hh, 0:w],
                                compare_op=mybir.AluOpType.is_ge,
                                fill=0.0, base=-128 * s,
                                pattern=[[1, w]], channel_multiplier=-1)
                    nc.tensor.matmul(
                        cps0, lhsT=vaug_sb[:, jb, 0, :], rhs=pt[:, 0, :],
                        start=(jb == 0), stop=(jb == njb - 1))
                    nc.tensor.matmul(
                        cps1, lhsT=vaug_sb[:, jb, 1, :], rhs=pt[:, 1, :],
                        start=(jb == 0), stop=(jb == njb - 1))
                    if jb % 2 == 1:
                        drain(1)

                ctxn = {0: normalize(cps0), 1: normalize(cps1)}

                # ---- pass B: head 2, alternating row groups ----
                cps2 = cpsp.tile([DK + 1, ICH_W], F32, tag="cps", space="PSUM")
                for grp in range(njb // 2):
                    st = stps.tile([128, 2, ICH_W], F32, tag="st",
                                   space="PSUM")
                    for jj in range(2):
                        jb = grp * 2 + jj
                        j0 = jb * JB_W
                        if jb % 2 == 0:
                            lhsT = qk_sb[0:64, 3, j0:j0 + JB_W]
                            rhs = qk_sb[0:64, 2, i0:i0 + ICH_W]
                        else:
                            lhsT = qk_sb[64:128, 2, j0:j0 + JB_W]
                            rhs = qk_sb[64:128, 3, i0:i0 + ICH_W]
                        nc.tensor.matmul(st[:, jj, :], lhsT=lhsT, rhs=rhs,
                                         start=True, stop=True)
                    pt = ptp.tile([128, 2, ICH_W], F16, tag="pt")
                    nc.scalar.activation(pt, st,
                                         mybir.ActivationFunctionType.Exp,
                                         bias=0.0, scale=1.0 / np.sqrt(DK))
                    for jj in range(2):
                        jb = grp * 2 + jj
                        s = jb - (njb - 4)
                        if s >= 0:
                            w = 128 * (s + 1)
                            nc.gpsimd.affine_select(
                                out=pt[:, jj, 0:w], in_=pt[:, jj, 0:w],
                                compare_op=mybir.AluOpType.is_ge,
                                fill=0.0, base=-128 * s,
                                pattern=[[1, w]], channel_multiplier=-1)
                    for jj in range(2):
                        jb = grp * 2 + jj
                        nc.tensor.matmul(
                            cps2, lhsT=vaug_sb[:, jb, 2, :], rhs=pt[:, jj, :],
                            start=(jb == 0), stop=(jb == njb - 1))
                    drain(1)

                ctxn[2] = normalize(cps2)

                # drain leftovers (proj j+1 must be fully emitted before the
                # next chunk's attention reads qk_sb at its own columns)
                drain(len(pending))
                pending.extend(outproj_thunks(ich, ctxn))

            drain(len(pending))

    nc.compile()
    return nc


def make_core_inputs(x_full, W_qkv, b_qkv, W_out, b, hg, t=T,
                     use_coll=USE_COLL):
    """Host-side input prep for core (b, hg): fp16 shard + permuted weights."""
    tpc = t // 4
    if use_coll:
        xs = np.asarray(x_full[b][hg * tpc:(hg + 1) * tpc], np.float32)
    else:
        xs = np.asarray(x_full[b], np.float32)
    x_in = np.ascontiguousarray(xs.T, np.float16).reshape(KT, 128, -1)

    heads = [hg * HPC + i for i in range(HPC)]
    # W_qkv last-dim layout: c = h*192 + s*64 + d  (s: 0=q 1=k 2=v)
    def cols(h, s):
        return slice(h * 192 + s * 64, h * 192 + s * 64 + 64)

    q = [np.asarray(W_qkv[:, cols(h, 0)]) for h in heads]
    k = [np.asarray(W_qkv[:, cols(h, 1)]) for h in heads]
    v = [np.asarray(W_qkv[:, cols(h, 2)]) for h in heads]
    bq = [np.asarray(b_qkv[cols(h, 0)]) for h in heads]

    wqk = np.concatenate([q[0], q[1], k[0], k[1], q[2], k[2], k[2], q[2]],
                         axis=1).astype(np.float16)
    z = np.zeros(64, np.float32)
    bqk = np.concatenate([bq[0], bq[1], z, z, bq[2], z, z, bq[2]]).astype(
        np.float32)
    wv = np.concatenate(v, axis=1).astype(np.float16)
    wout = np.concatenate(
        [np.asarray(W_out[h * DK:(h + 1) * DK, :]) for h in heads],
        axis=0).astype(np.float16)
    return {
        "x": np.ascontiguousarray(x_in),
        "wqk": np.ascontiguousarray(wqk),
        "bqk": np.ascontiguousarray(bqk),
        "wv": np.ascontiguousarray(wv),
        "wout": np.ascontiguousarray(wout),
    }


_CACHE = {}


def _get_program(t=T, use_coll=USE_COLL):
    key = (t, use_coll)
    if key not in _CACHE:
        _CACHE[key] = build_program(t, use_coll)
    return _CACHE[key]


def run_cores(inputs, t=T, trace=False):
    nc = _get_program(t)
    x = np.asarray(inputs["x"], np.float32)
    in_maps = []
    for core in range(N_CORES):
        b, hg = core // 4, core % 4
        in_maps.append(make_core_inputs(x, inputs["W_qkv"], inputs["b_qkv"],
                                        inputs["W_out"], b, hg, t=t))
    res = run_bass_kernel_spmd(nc, in_maps, list(range(N_CORES)), trace=trace)
    return res


def gather(inputs, results, t=T, use_coll=USE_COLL):
    b_qkv = np.asarray(inputs["b_qkv"], np.float32)
    W_out = np.asarray(inputs["W_out"], np.float32)
    b_out = np.asarray(inputs["b_out"], np.float32)
    bv = np.concatenate([b_qkv[h * 192 + 128:h * 192 + 192] for h in range(H)])
    fold = bv @ W_out + b_out                      # [D]
    n_ich = t // ICH_W
    out = np.zeros((B, t, D), np.float32)
    for core in range(N_CORES):
        b, c = core // 4, core % 4
        r = np.asarray(results[core]["out"], np.float16).astype(np.float32)
        if use_coll:
            for ich in range(n_ich):
                r0 = ich * ICH_W + c * JB_W
                out[b, r0:r0 + JB_W, :] = r[ich]
        else:
            out[b] += r
    out += fold[None, None, :]
    return out


def kernel(**inputs):
    res = run_cores(inputs)
    return gather(inputs, res.results)


if __name__ == "__main__":
    # smoke test with random data
    rng = np.random.default_rng(0)
    inputs = {
        "x": rng.standard_normal((B, T, D), dtype=np.float32),
        "mask": np.triu(np.ones((T, T), dtype=bool), k=1),
        "W_qkv": (rng.standard_normal((D, 3 * D), dtype=np.float32)
                  / np.sqrt(D)),
        "b_qkv": rng.standard_normal(3 * D).astype(np.float32) * 0.02,
        "W_out": (rng.standard_normal((D, D), dtype=np.float32)
                  / np.sqrt(D)),
        "b_out": rng.standard_normal(D).astype(np.float32) * 0.02,
    }
    out = kernel(**inputs)
    print(out.shape, out.dtype)


# revision 11
# speedup vs baseline: 1.3025x; 1.0324x over previous
"""Trainium2 Bass kernel for causal multi-head attention.

Problem: B=2, T=4096, D=768, H=12 heads, d_k=64, causal mask.
Sharding: 8 cores = 2 batches x 4 head-groups (3 heads each).

v2 design:
- All inputs shipped fp16 (x pre-transposed on host into [6, 128, T'] chunks).
- x is uploaded T-sharded (each core gets a distinct 1024-token slice of its
  batch) and AllGather'd on-device across the 4-core batch group, cutting
  host->device x bytes 4x vs replicating.
- All matmuls run fp16 operands (1 cyc/row + fast weight load), fp32 PSUM.
- Attention uses transposed scores (S^T = k q^T) so softmax statistics land
  matmul-friendly with no P-transposes; denominators via an appended
  ones-column in v (row 64 of the PV accumulation).
- Softmax normalization via reciprocal_approx_fast on a broadcast tile
  (the old single-lane PSUM reciprocal was 3.3us per call).
- Projections for chunk i+1 and the deferred output projection of chunk i-1
  are interleaved into attention's matmul stream as filler, keeping TensorE
  dense so the HAM clock gate stays at full rate.
- Partial outputs are ReduceScatter'd on-device (fp16) per 512-token chunk,
  overlapped with compute; each core returns a disjoint [8, 128, 768] slab,
  cutting device->host bytes 16x vs fp32 replicated partials.
- Host folds the v-bias through W_out and adds b_out. k-bias is dropped
  (softmax is invariant to per-query score shifts).

Self-contained: hardcodes all shapes; only imports the concourse runtime.
"""

import sys

sys.path.insert(0, "/opt/trn_rl_repo")

from contextlib import ExitStack

import numpy as np

import concourse.bass as bass
import concourse.mybir as mybir
import concourse.tile as tile
from concourse import bacc
from concourse.bass_utils import run_bass_kernel_spmd

F16 = mybir.dt.float16
F32 = mybir.dt.float32

B, T, D = 2, 4096, 768
H, DK = 12, 64
HPC = 3          # heads per core
N_CORES = 8
ICH_W = 512      # i-chunk width (queries per outer step)
JB_W = 128       # j-block width (keys per matmul)
KT = D // 128    # contraction tiles for projections
VW = HPC * DK    # v projection width
GROUPS = [[0, 1, 2, 3], [4, 5, 6, 7]]

USE_COLL = True  # AllGather x + ReduceScatter out on-device


def build_program(t=T, use_coll=USE_COLL):
    """Build the SPMD Bass program for one core (all cores identical)."""
    n_ich = t // ICH_W
    n_tch = t // 128
    tpc = t // 4                  # tokens per core in the x shard (AG mode)

    nc = bacc.Bacc("TRN2", target_bir_lowering=False, debug=False,
                   num_devices=N_CORES)

    sw = t // 16                  # strip width (4 strips per core shard)
    if use_coll:
        # core c's local strip s = global strip 4s+c (so AG#s delivers the
        # 4 consecutive global strips 4s..4s+3 = chunks 2s, 2s+1)
        x_d = nc.dram_tensor("x", [4, KT, 128, sw], F16,
                             kind="ExternalInput").ap()
    else:
        x_d = nc.dram_tensor("x", [KT, 128, t], F16,
                             kind="ExternalInput").ap()
    # qk projection weights, 4 chunks of 128 output channels:
    # ch0=[q0|q1] ch1=[k0|k1] ch2=[q2|k2] ch3=[k2|q2]
    wqk_d = nc.dram_tensor("wqk", [D, 512], F16, kind="ExternalInput").ap()
    bqk_d = nc.dram_tensor("bqk", [512], F32, kind="ExternalInput").ap()
    wv_d = nc.dram_tensor("wv", [D, VW], F16, kind="ExternalInput").ap()
    wout_d = nc.dram_tensor("wout", [VW, D], F16, kind="ExternalInput").ap()
    if use_coll:
        out_d = nc.dram_tensor("out", [n_ich, JB_W, D], F16,
                               kind="ExternalOutput").ap()
    else:
        out_d = nc.dram_tensor("out", [t, D], F16, kind="ExternalOutput").ap()

    with tile.TileContext(nc) as tc, ExitStack() as top:
        consts = top.enter_context(tc.tile_pool(name="consts", bufs=1))
        persist = top.enter_context(tc.tile_pool(name="persist", bufs=1))

        wqk_sb = consts.tile([128, KT, 512], F16)
        nc.sync.dma_start(out=wqk_sb,
                          in_=wqk_d.rearrange("(kt p) c -> p kt c", p=128))
        bqk_sb = consts.tile([128, 4], F32)
        nc.sync.dma_start(out=bqk_sb, in_=bqk_d.rearrange("(ch p) -> p ch",
                                                          p=128))
        wv_sb = consts.tile([128, KT, VW], F16)
        nc.sync.dma_start(out=wv_sb,
                          in_=wv_d.rearrange("(kt p) c -> p kt c", p=128))
        wout_sb = consts.tile([64, HPC, D], F16)
        nc.sync.dma_start(out=wout_sb,
                          in_=wout_d.rearrange("(h p) m -> p h m", p=64))

        # persistent activations: q^T/k^T chunks and v (+ones col)
        qk_sb = persist.tile([128, 4, t], F16)
        vaug_sb = persist.tile([128, n_tch, HPC, DK + 1], F16)
        nc.vector.memset(vaug_sb[:, :, :, DK:DK + 1], 1.0)

        if use_coll:
            dram = top.enter_context(
                tc.tile_pool(name="dram", bufs=1, space="DRAM"))
            xb = dram.tile([4, KT, 128, sw], F16)
            xg = dram.tile([4, 4, KT, 128, sw], F16)
            ob = dram.tile([t, D], F16)
            rsb = dram.tile([n_ich, JB_W, D], F16)
            for s in range(4):
                nc.gpsimd.dma_start(xb[s], x_d[s])
                nc.gpsimd.collective_compute(
                    "AllGather", mybir.AluOpType.bypass,
                    replica_groups=GROUPS,
                    ins=[xb[s].opt()], outs=[xg[s].opt()])

        with tc.tile_pool(name="xtp", bufs=2) as xtp, \
             tc.tile_pool(name="stps", bufs=2, space="PSUM") as stps, \
             tc.tile_pool(name="cpsp", bufs=3, space="PSUM") as cpsp, \
             tc.tile_pool(name="fillp", bufs=1, space="PSUM") as fillp, \
             tc.tile_pool(name="ptp", bufs=3) as ptp, \
             tc.tile_pool(name="ctxp", bufs=6) as ctxp, \
             tc.tile_pool(name="smp", bufs=4) as smp, \
             tc.tile_pool(name="outp", bufs=2) as outp:

            def load_xt(j):
                xt = xtp.tile([128, KT, ICH_W], F16, tag="xt")
                i0 = j * ICH_W
                if use_coll:
                    s, r0 = j // 2, 2 * (j % 2)
                    hw_ = ICH_W // 2
                    for rr in range(2):
                        nc.sync.dma_start(
                            out=xt[:, :, rr * hw_:(rr + 1) * hw_],
                            in_=xg[s, r0 + rr].rearrange("kt p w -> p kt w"))
                else:
                    src = x_d[:, :, i0:i0 + ICH_W].rearrange("kt p w -> p kt w")
                    nc.sync.dma_start(out=xt, in_=src)
                return xt

            def proj_thunks(j, xt):
                """Filler thunks computing chunk j's qkv projections."""
                i0 = j * ICH_W
                ths = []
                for ch in range(4):
                    def th(ch=ch):
                        qps = fillp.tile([128, 512], F32, tag="fill",
                                         space="PSUM")
                        for kt in range(KT):
                            nc.tensor.matmul(
                                qps,
                                lhsT=wqk_sb[:, kt, ch * 128:(ch + 1) * 128],
                                rhs=xt[:, kt, :],
                                start=(kt == 0), stop=(kt == KT - 1))
                        nc.vector.tensor_scalar_add(
                            qk_sb[:, ch, i0:i0 + ICH_W], qps,
                            bqk_sb[:, ch:ch + 1])
                    ths.append(th)
                for tl in range(4):
                    def th(tl=tl):
                        vps = fillp.tile([128, 512], F32, tag="fill",
                                         space="PSUM")
                        for kt in range(KT):
                            nc.tensor.matmul(
                                vps[:, 0:VW],
                                lhsT=xt[:, kt, tl * 128:(tl + 1) * 128],
                                rhs=wv_sb[:, kt, :],
                                start=(kt == 0), stop=(kt == KT - 1))
                        nc.vector.tensor_copy(
                            vaug_sb[:, j * 4 + tl, :, 0:DK],
                            vps[:, 0:VW].rearrange("p (h d) -> p h d", h=HPC))
                    ths.append(th)
                return ths

            def outproj_thunks(j, ctxn, split_rs=False):
                """Filler thunks for chunk j's output projection (+ its RS)."""
                i0 = j * ICH_W
                ths = []
                osbs = {}
                for tsub in range(4):
                    for mh, (m0, m1) in enumerate(((0, 384), (384, D))):
                        def th(tsub=tsub, mh=mh, m0=m0, m1=m1):
                            if mh == 0:
                                osbs[tsub] = outp.tile([128, D], F16,
                                                       tag="osb", name="osb")
                            osb = osbs[tsub]
                            ops = fillp.tile([128, 512], F32, tag="fill",
                                             space="PSUM")
                            for h in range(HPC):
                                nc.tensor.matmul(
                                    ops[:, 0:384],
                                    lhsT=ctxn[h][:,
                                                 tsub * 128:(tsub + 1) * 128],
                                    rhs=wout_sb[:, h, m0:m1],
                                    start=(h == 0), stop=(h == HPC - 1))
                            nc.vector.tensor_copy(osb[:, m0:m1], ops[:, 0:384])
                            if mh == 1:
                                dst = ob if use_coll else out_d
                                nc.sync.dma_start(
                                    out=dst[i0 + tsub * 128:
                                            i0 + (tsub + 1) * 128, :],
                                    in_=osb)
                                if use_coll and split_rs:
                                    # tail chunk: per-tsub RS so the last
                                    # collective only covers 128 rows
                                    nc.gpsimd.collective_compute(
                                        "ReduceScatter", mybir.AluOpType.add,
                                        replica_groups=GROUPS,
                                        ins=[ob[i0 + tsub * 128:
                                                i0 + (tsub + 1) * 128,
                                                :].opt()],
                                        outs=[rsb[j, tsub * 32:
                                                  (tsub + 1) * 32, :].opt()])
                                    nc.gpsimd.dma_start(
                                        out_d[j, tsub * 32:(tsub + 1) * 32, :],
                                        rsb[j, tsub * 32:(tsub + 1) * 32, :])
                        ths.append(th)
                if use_coll and not split_rs:
                    def th_rs(j=j):
                        nc.gpsimd.collective_compute(
                            "ReduceScatter", mybir.AluOpType.add,
                            replica_groups=GROUPS,
                            ins=[ob[j * ICH_W:(j + 1) * ICH_W, :].opt()],
                            outs=[rsb[j].opt()])
                        nc.gpsimd.dma_start(out_d[j], rsb[j])
                    ths.append(th_rs)
                return ths

            def normalize(cps):
                den = smp.tile([1, ICH_W], F32, tag="den")
                nc.vector.tensor_copy(den, cps[DK:DK + 1, :])
                rec = smp.tile([1, ICH_W], F32, tag="rec")
                nc.vector.reciprocal_approx_fast(out=rec, in_=den)
                rb = smp.tile([64, ICH_W], F32, tag="rb")
                nc.gpsimd.partition_broadcast(rb, rec)
                cn = ctxp.tile([64, ICH_W], F16, tag="ctxn")
                nc.vector.tensor_mul(cn, cps[0:DK, :], rb)
                return cn

            pending = []

            def drain(k):
                for _ in range(k):
                    if pending:
                        pending.pop(0)()

            xt = load_xt(0)
            for th in proj_thunks(0, xt):
                th()

            for ich in range(n_ich):
                i0 = ich * ICH_W
                njb = (i0 + ICH_W) // JB_W
                if ich + 1 < n_ich:
                    xt = load_xt(ich + 1)
                    pending.extend(proj_thunks(ich + 1, xt))

                # ---- pass A: heads 0 and 1, row-group paired ----
                cps0 = cpsp.tile([DK + 1, ICH_W], F32, tag="cps", space="PSUM")
                cps1 = cpsp.tile([DK + 1, ICH_W], F32, tag="cps", space="PSUM")
                for jb in range(njb):
                    j0 = jb * JB_W
                    st = stps.tile([128, 2, ICH_W], F32, tag="st",
                                   space="PSUM")
                    nc.tensor.matmul(
                        st[:, 0, :], lhsT=qk_sb[0:64, 1, j0:j0 + JB_W],
                        rhs=qk_sb[0:64, 0, i0:i0 + ICH_W],
                        start=True, stop=True)
                    nc.tensor.matmul(
                        st[:, 1, :], lhsT=qk_sb[64:128, 1, j0:j0 + JB_W],
                        rhs=qk_sb[64:128, 0, i0:i0 + ICH_W],
                        start=True, stop=True)
                    pt = ptp.tile([128, 2, ICH_W], F16, tag="pt")
                    nc.scalar.activation(pt, st,
                                         mybir.ActivationFunctionType.Exp,
                                         bias=0.0, scale=1.0 / np.sqrt(DK))
                    s = jb - (njb - 4)          # diag position if >= 0
                    if s >= 0:
                        w = 128 * (s + 1)
                        for hh in range(2):
                            nc.gpsimd.affine_select(
                                out=pt[:, hh, 0:w], in_=pt[:, hh, 0:w],
                                compare_op=mybir.AluOpType.is_ge,
                                fill=0.0, base=-128 * s,
                                pattern=[[1, w]], channel_multiplier=-1)
                    nc.tensor.matmul(
                        cps0, lhsT=vaug_sb[:, jb, 0, :], rhs=pt[:, 0, :],
                        start=(jb == 0), stop=(jb == njb - 1))
                    nc.tensor.matmul(
                        cps1, lhsT=vaug_sb[:, jb, 1, :], rhs=pt[:, 1, :],
                        start=(jb == 0), stop=(jb == njb - 1))
                    if jb % 2 == 1:
                        drain(1)

                ctxn = {0: normalize(cps0), 1: normalize(cps1)}

                # ---- pass B: head 2, alternating row groups ----
                cps2 = cpsp.tile([DK + 1, ICH_W], F32, tag="cps", space="PSUM")
                for grp in range(njb // 2):
                    st = stps.tile([128, 2, ICH_W], F32, tag="st",
                                   space="PSUM")
                    for jj in range(2):
                        jb = grp * 2 + jj
                        j0 = jb * JB_W
                        if jb % 2 == 0:
                            lhsT = qk_sb[0:64, 3, j0:j0 + JB_W]
                            rhs = qk_sb[0:64, 2, i0:i0 + ICH_W]
                        else:
                            lhsT = qk_sb[64:128, 2, j0:j0 + JB_W]
                            rhs = qk_sb[64:128, 3, i0:i0 + ICH_W]
                        nc.tensor.matmul(st[:, jj, :], lhsT=lhsT, rhs=rhs,
                                         start=True, stop=True)
                    pt = ptp.tile([128, 2, ICH_W], F16, tag="pt")
                    nc.scalar.activation(pt, st,
                                         mybir.ActivationFunctionType.Exp,
                                         bias=0.0, scale=1.0 / np.sqrt(DK))
                    for jj in range(2):
                        jb = grp * 2 + jj
                        s = jb - (njb - 4)
                        if s >= 0:
                            w = 128 * (s + 1)
                            nc.gpsimd.affine_select(
                                out=pt[:, jj, 0:w], in_=pt[:, jj, 0:w],
                                compare_op=mybir.AluOpType.is_ge,
                                fill=0.0, base=-128 * s,
                                pattern=[[1, w]], channel_multiplier=-1)
                    for jj in range(2):
                        jb = grp * 2 + jj
                        nc.tensor.matmul(
                            cps2, lhsT=vaug_sb[:, jb, 2, :], rhs=pt[:, jj, :],
                            start=(jb == 0), stop=(jb == njb - 1))
                    drain(1)

                ctxn[2] = normalize(cps2)

                # drain leftovers (proj j+1 must be fully emitted before the
                # next chunk's attention reads qk_sb at its own columns)
                drain(len(pending))
                pending.extend(outproj_thunks(ich, ctxn,
                                              split_rs=(ich == n_ich - 1)))

            drain(len(pending))

    nc.compile()
    return nc


def make_core_inputs(x_full, W_qkv, b_qkv, W_out, b, hg, t=T,
                     use_coll=USE_COLL):
    """Host-side input prep for core (b, hg): fp16 shard + permuted weights."""
    if use_coll:
        sw = t // 16
        x_in = np.stack([
            np.ascontiguousarray(
                np.asarray(x_full[b][(4 * s + hg) * sw:(4 * s + hg + 1) * sw],
                           np.float32).T, np.float16).reshape(KT, 128, sw)
            for s in range(4)])
    else:
        xs = np.asarray(x_full[b], np.float32)
        x_in = np.ascontiguousarray(xs.T, np.float16).reshape(KT, 128, -1)

    heads = [hg * HPC + i for i in range(HPC)]
    # W_qkv last-dim layout: c = h*192 + s*64 + d  (s: 0=q 1=k 2=v)
    def cols(h, s):
        return slice(h * 192 + s * 64, h * 192 + s * 64 + 64)

    q = [np.asarray(W_qkv[:, cols(h, 0)]) for h in heads]
    k = [np.asarray(W_qkv[:, cols(h, 1)]) for h in heads]
    v = [np.asarray(W_qkv[:, cols(h, 2)]) for h in heads]
    bq = [np.asarray(b_qkv[cols(h, 0)]) for h in heads]

    wqk = np.concatenate([q[0], q[1], k[0], k[1], q[2], k[2], k[2], q[2]],
                         axis=1).astype(np.float16)
    z = np.zeros(64, np.float32)
    bqk = np.concatenate([bq[0], bq[1], z, z, bq[2], z, z, bq[2]]).astype(
        np.float32)
    wv = np.concatenate(v, axis=1).astype(np.float16)
    wout = np.concatenate(
        [np.asarray(W_out[h * DK:(h + 1) * DK, :]) for h in heads],
        axis=0).astype(np.float16)
    return {
        "x": np.ascontiguousarray(x_in),
        "wqk": np.ascontiguousarray(wqk),
        "bqk": np.ascontiguousarray(bqk),
        "wv": np.ascontiguousarray(wv),
        "wout": np.ascontiguousarray(wout),
    }


_CACHE = {}


def _get_program(t=T, use_coll=USE_COLL):
    key = (t, use_coll)
    if key not in _CACHE:
        _CACHE[key] = build_program(t, use_coll)
    return _CACHE[key]


def run_cores(inputs, t=T, trace=False):
    nc = _get_program(t)
    x = np.asarray(inputs["x"], np.float32)
    in_maps = []
    for core in range(N_CORES):
        b, hg = core // 4, core % 4
        in_maps.append(make_core_inputs(x, inputs["W_qkv"], inputs["b_qkv"],
                                        inputs["W_out"], b, hg, t=t))
    res = run_bass_kernel_spmd(nc, in_maps, list(range(N_CORES)), trace=trace)
    return res


def gather(inputs, results, t=T, use_coll=USE_COLL):
    b_qkv = np.asarray(inputs["b_qkv"], np.float32)
    W_out = np.asarray(inputs["W_out"], np.float32)
    b_out = np.asarray(inputs["b_out"], np.float32)
    bv = np.concatenate([b_qkv[h * 192 + 128:h * 192 + 192] for h in range(H)])
    fold = bv @ W_out + b_out                      # [D]
    n_ich = t // ICH_W
    out = np.zeros((B, t, D), np.float32)
    for core in range(N_CORES):
        b, c = core // 4, core % 4
        r = np.asarray(results[core]["out"], np.float16).astype(np.float32)
        if use_coll:
            for ich in range(n_ich - 1):
                r0 = ich * ICH_W + c * JB_W
                out[b, r0:r0 + JB_W, :] = r[ich]
            # last chunk used per-tsub split RS: row 32k+m <-> token
            # (n_ich-1)*ICH_W + 128k + 32c + m
            i0 = (n_ich - 1) * ICH_W
            for k in range(4):
                r0 = i0 + k * JB_W + c * 32
                out[b, r0:r0 + 32, :] = r[n_ich - 1][k * 32:(k + 1) * 32]
        else:
            out[b] += r
    out += fold[None, None, :]
    return out


def kernel(**inputs):
    res = run_cores(inputs)
    return gather(inputs, res.results)


if __name__ == "__main__":
    # smoke test with random data
    rng = np.random.default_rng(0)
    inputs = {
        "x": rng.standard_normal((B, T, D), dtype=np.float32),
        "mask": np.triu(np.ones((T, T), dtype=bool), k=1),
        "W_qkv": (rng.standard_normal((D, 3 * D), dtype=np.float32)
                  / np.sqrt(D)),
        "b_qkv": rng.standard_normal(3 * D).astype(np.float32) * 0.02,
        "W_out": (rng.standard_normal((D, D), dtype=np.float32)
                  / np.sqrt(D)),
        "b_out": rng.standard_normal(D).astype(np.float32) * 0.02,
    }
    out = kernel(**inputs)
    print(out.shape, out.dtype)
